# revision 3
# baseline (speedup 1.0000x reference)
"""Trainium2 Bass kernel for ConvBlock: 3x3 conv (64->128ch) + sync-BN + ReLU6.

Strategy: data-parallel over batch (4 images/core on 8 cores).
The host zero-pads x to [NB, 64, 58, 58]. SBUF tile XR holds the padded x on
partitions 0-63 and the same data shifted up one row on partitions 64-127.
Conv is 6 fp32r matmuls per 8-row PSUM tile, all full-width N=448:
  - 3x K=128 "row-pair" matmuls (taps kh in {0,1}), one per kw, where the kw
    shift is a free-dim AP offset into the padded rows;
  - 3x K=64 matmuls for the kh=2 taps reading the shifted half one row down.
BatchNorm batch stats via bn_stats/bn_aggr per core, cross-core AllReduce of
(mean, E[y^2]) (1KB), then fused (y*inv + shift) -> ReLU -> min(6) and DMA out.
"""

import sys

sys.path.insert(0, "/root/.axon_site/_ro/trn_rl_repo")

import numpy as np

# ---- hardcoded problem config ----
NB_TOTAL, CIN, H, W = 32, 64, 56, 56
HP, WP = H + 2, W + 2  # host-padded spatial dims
COUT = 128
NCORES = 8
NB = NB_TOTAL // NCORES  # 4 images per core
BN_EPS = 1e-5
ACT_THRES = 6.0
R = 8  # psum tile rows
NT = H // R  # 7 tiles per image
NTILE = NB * NT  # 28 psum tiles per core

_cache = {}


def _build():
    if "nc" in _cache:
        return _cache["nc"]

    import concourse.tile as tile
    from concourse import bacc, mybir

    f32 = mybir.dt.float32
    f32r = mybir.dt.float32r

    nc = bacc.Bacc("TRN2", target_bir_lowering=False, debug=False, num_devices=NCORES)

    x_d = nc.dram_tensor("x", [NB, CIN, HP, WP], f32r, kind="ExternalInput")
    w_d = nc.dram_tensor("w", [128, 6, 128], f32r, kind="ExternalInput")
    g_d = nc.dram_tensor("gamma", [COUT, 1], f32, kind="ExternalInput")
    b_d = nc.dram_tensor("beta", [COUT, 1], f32, kind="ExternalInput")
    o_d = nc.dram_tensor("out", [NB, COUT, H, W], f32, kind="ExternalOutput")

    with tile.TileContext(nc) as tc:
        with (
            tc.tile_pool(name="big", bufs=1) as big,
            tc.tile_pool(name="small", bufs=1) as small,
            tc.tile_pool(name="psum", bufs=8, space="PSUM") as psum,
            tc.tile_pool(name="dram", bufs=1, space="DRAM") as dram,
        ):
            XR = big.tile([128, NB, HP, WP], f32r, tag="XR")
            Y = big.tile([COUT, NB, H, W], f32, tag="Y")
            WT = small.tile([128, 6, 128], f32r, tag="WT")
            GM = small.tile([COUT, 1], f32, tag="GM")
            BT = small.tile([COUT, 1], f32, tag="BT")
            S6 = small.tile([COUT, NTILE, 6], f32, tag="S6")

            # weights + bn params
            nc.sync.dma_start(WT[:], w_d[:])
            nc.sync.dma_start(GM[:], g_d[:])
            nc.sync.dma_start(BT[:], b_d[:])

            # x loads: top half = padded x, bottom half = same shifted up one row
            for n in range(NB):
                nc.sync.dma_start(XR[0:64, n, :, :], x_d[n])
                nc.sync.dma_start(XR[64:128, n, 0 : HP - 1, :], x_d[n, :, 1:HP, :])

            # conv: 6 matmuls per psum tile, all N = R*W = 448
            for n in range(NB):
                for t in range(NT):
                    r0 = t * R
                    pt = psum.tile([COUT, R, W], f32, tag="pt")
                    # row-pair taps (kh=0,1), kw = 1, 0, 2; center opens bank
                    for j, kw in enumerate((1, 0, 2)):
                        nc.tensor.matmul(
                            pt[:, :, :],
                            WT[:, kw, :],
                            XR[:, n, r0 : r0 + R, kw : kw + W],
                            start=(j == 0),
                            stop=False,
                        )
                    # kh=2 taps via the shifted half one row down (K=64)
                    for j, kw in enumerate((0, 1, 2)):
                        nc.tensor.matmul(
                            pt[:, :, :],
                            WT[64:128, 3 + kw, :],
                            XR[64:128, n, r0 + 1 : r0 + 1 + R, kw : kw + W],
                            start=False,
                            stop=(j == 2),
                        )

                    ti = n * NT + t
                    nc.scalar.copy(Y[:, n, r0 : r0 + R, :], pt[:, :, :])
                    nc.vector.bn_stats(
                        S6[:, ti, :], pt[:].rearrange("p a b -> p (a b)")
                    )

            # per-core (mean, var) then (mean, E[y^2]) for the all-reduce
            S2 = small.tile([COUT, 2], f32, tag="S2")
            nc.vector.bn_aggr(S2[:], S6[:].rearrange("p a b -> p (a b)"))
            ARin = small.tile([COUT, 2], f32, tag="ARin")
            TMP = small.tile([COUT, 4], f32, tag="TMP")
            nc.vector.tensor_copy(ARin[:, 0:1], S2[:, 0:1])
            nc.vector.tensor_mul(TMP[:, 0:1], S2[:, 0:1], S2[:, 0:1])
            nc.vector.tensor_add(ARin[:, 1:2], S2[:, 1:2], TMP[:, 0:1])

            cc_in = dram.tile([COUT, 2], f32)
            cc_out = dram.tile([COUT, 2], f32)
            nc.sync.dma_start(cc_in[:], ARin[:])
            nc.gpsimd.collective_compute(
                "AllReduce",
                mybir.AluOpType.add,
                ins=[cc_in.opt()],
                outs=[cc_out.opt()],
                replica_groups=[list(range(NCORES))],
            )
            ARout = small.tile([COUT, 2], f32, tag="ARout")
            nc.sync.dma_start(ARout[:], cc_out[:])

            # mean = sum/8 ; E2 = sum/8 ; var = E2 - mean^2
            MEAN = small.tile([COUT, 1], f32, tag="MEAN")
            INV = small.tile([COUT, 1], f32, tag="INV")
            SHIFT = small.tile([COUT, 1], f32, tag="SHIFT")
            inv_n = 1.0 / NCORES
            EPS = small.tile([COUT, 1], f32, tag="EPS")
            nc.vector.memset(EPS[:], BN_EPS)
            nc.scalar.mul(MEAN[:], ARout[:, 0:1], inv_n)
            # TMP0 = E2 + eps, TMP1 = mean^2, TMP2 = var + eps
            nc.scalar.activation(
                TMP[:, 0:1],
                ARout[:, 1:2],
                mybir.ActivationFunctionType.Identity,
                bias=EPS[:, 0:1],
                scale=inv_n,
            )
            nc.vector.tensor_mul(TMP[:, 1:2], MEAN[:], MEAN[:])
            nc.vector.tensor_sub(TMP[:, 2:3], TMP[:, 0:1], TMP[:, 1:2])
            nc.scalar.sqrt(TMP[:, 3:4], TMP[:, 2:3])
            nc.vector.reciprocal(TMP[:, 0:1], TMP[:, 3:4])
            nc.vector.tensor_mul(INV[:], TMP[:, 0:1], GM[:])
            # shift = beta - mean*inv
            nc.vector.tensor_mul(TMP[:, 1:2], MEAN[:], INV[:])
            nc.vector.tensor_sub(SHIFT[:], BT[:], TMP[:, 1:2])

            # normalize + ReLU6 + store, per image
            for n in range(NB):
                nc.scalar.activation(
                    Y[:, n],
                    Y[:, n],
                    mybir.ActivationFunctionType.Relu,
                    bias=SHIFT[:, 0:1],
                    scale=INV[:, 0:1],
                )
                nc.vector.tensor_scalar_min(Y[:, n], Y[:, n], ACT_THRES)
                nc.sync.dma_start(o_d[n], Y[:, n])

    nc.compile()
    _cache["nc"] = nc
    return nc


def _prep_inputs(x, w_blocks, gamma, beta):
    p, q, mb, _ = w_blocks.shape
    w = np.transpose(w_blocks, (0, 2, 1, 3)).reshape(p * mb, q * mb)
    w = w[:COUT, : CIN * 9].reshape(COUT, CIN, 3, 3).astype(np.float32)
    WT = np.zeros((128, 6, 128), np.float32)
    for kw in range(3):
        WT[0:64, kw, :] = w[:, :, 0, kw].T
        WT[64:128, kw, :] = w[:, :, 1, kw].T
        WT[64:128, 3 + kw, :] = w[:, :, 2, kw].T
    g = np.asarray(gamma, np.float32).reshape(COUT, 1)
    b = np.asarray(beta, np.float32).reshape(COUT, 1)
    x = np.asarray(x, np.float32)
    xp = np.zeros((NB_TOTAL, CIN, HP, WP), np.float32)
    xp[:, :, 1 : H + 1, 1 : W + 1] = x
    in_maps = [
        {
            "x": np.ascontiguousarray(xp[i * NB : (i + 1) * NB]),
            "w": WT,
            "gamma": g,
            "beta": b,
        }
        for i in range(NCORES)
    ]
    return in_maps


def _run(x, w_blocks, gamma, beta, trace=False):
    from concourse.bass_utils import run_bass_kernel_spmd

    nc = _build()
    in_maps = _prep_inputs(x, w_blocks, gamma, beta)
    res = run_bass_kernel_spmd(
        nc, in_maps, core_ids=list(range(NCORES)), trace=trace
    )
    out = np.concatenate([res.results[i]["out"] for i in range(NCORES)], axis=0)
    return out, res


def kernel(x, w_blocks, gamma, beta):
    out, _ = _run(x, w_blocks, gamma, beta, trace=False)
    return out


def run_traced(x, w_blocks, gamma, beta):
    out, res = _run(x, w_blocks, gamma, beta, trace=True)
    return out, res
